# revision 11
# baseline (speedup 1.0000x reference)
"""DualAttention Trainium2 kernel.

Sharding: 8 cores = 4 samples x 2 query-halves. Per core the sample image is
"rolled" by the half offset (host-side, with correct zero padding), so every
core runs the identical program on its first 2048 query positions; attention
over key positions is permutation-invariant, so convs/attention on the rolled
image give the true result for the core's half.

Per-core pipeline (all matmuls fp32r at full PE rate):
  qk conv (9-tap, fused q+k, co=128) -> scores T[j,i] = k^T q (row-tiled pairs)
  -> exp (ACT, no max-subtraction: |scores| <= ~8) -> U = vT @ E accumulated in
  PSUM over j-tiles (flash style, i split in 2 halves) ; denominator = column
  sums of E accumulated on DVE -> local = U * (1/denom) ; SE branch + glob ;
  1x1 fuse conv with folded biases.
"""

import sys

sys.path.insert(0, "/opt/trn_rl_repo")

import numpy as np

import concourse.bass as bass
import concourse.mybir as mybir
import concourse.tile as tile
from concourse import bacc
from concourse.bass_utils import run_bass_kernel_spmd
from concourse.masks import make_identity

f32 = mybir.dt.float32
f32r = mybir.dt.float32r
AF = mybir.ActivationFunctionType

C = 256
CT = 2          # channel tiles of 128
Cr = 64
H = W = 64
HW = H * W      # 4096
HWh = 2048      # query positions per core
JT = 32         # key-position tiles of 128
IH = 2          # i halves of 1024
ICH = 2         # 512-chunks per i half
N_CORES = 8

_compiled = None


def _build(debug=False):
    nc = bacc.Bacc("TRN2", target_bir_lowering=False, debug=False,
                   num_devices=N_CORES)

    xp_d = nc.declare_dram_parameter("xp", [C, 68 * 66], f32r, isOutput=False)
    wqkt_d = nc.declare_dram_parameter("wqkt", [18, 128, 128], f32r, isOutput=False)
    wvt_d = nc.declare_dram_parameter("wvt", [18, 128, 256], f32r, isOutput=False)
    bqk_d = nc.declare_dram_parameter("bqk", [128, 1], f32, isOutput=False)
    fc1t_d = nc.declare_dram_parameter("fc1t", [2, 128, 16], f32, isOutput=False)
    fc1b_d = nc.declare_dram_parameter("fc1b", [16, 1], f32, isOutput=False)
    fc2t_d = nc.declare_dram_parameter("fc2t", [2, 16, 128], f32, isOutput=False)
    fc2bn_d = nc.declare_dram_parameter("fc2bn", [2, 128, 1], f32, isOutput=False)
    fuset_d = nc.declare_dram_parameter("fuset", [4, 2, 128, 128], f32r, isOutput=False)
    fuseb_d = nc.declare_dram_parameter("fuseb", [2, 128, 1], f32, isOutput=False)
    out_d = nc.declare_dram_parameter("out", [2, 128, HWh], f32, isOutput=True)
    if debug:
        qk_dbg = nc.declare_dram_parameter("qk_dbg", [128, HW], f32, isOutput=True)
        vt_dbg = nc.declare_dram_parameter("vt_dbg", [128, JT * 256], f32, isOutput=True)
        dacc_dbg = nc.declare_dram_parameter("dacc_dbg", [128, HWh], f32, isOutput=True)
        recb_dbg = nc.declare_dram_parameter("recb_dbg", [128, HWh], f32, isOutput=True)
        usb_dbg = nc.declare_dram_parameter("usb_dbg", [2, 128, HWh], f32, isOutput=True)
        glob_dbg = nc.declare_dram_parameter("glob_dbg", [2, 128, HWh], f32, isOutput=True)

    with tile.TileContext(nc) as tc, \
         nc.allow_low_precision(reason="fp32r is 4-byte storage; rounding only"):
        with tc.tile_pool(name="pw", bufs=1) as pw:
            # persistent tiles
            wqkt = pw.tile([128, 18, 128], f32r)
            nc.sync.dma_start(wqkt[:], wqkt_d[:].rearrange("t p m -> p t m"))
            wvt = pw.tile([128, 18, 256], f32r)
            nc.sync.dma_start(wvt[:], wvt_d[:].rearrange("t p m -> p t m"))
            fuset = pw.tile([128, 8, 128], f32r)
            nc.sync.dma_start(
                fuset[:].rearrange("p (k m) f -> p k m f", k=4),
                fuset_d[:].rearrange("k m p f -> p k m f"),
            )
            bqk = pw.tile([128, 1], f32)
            nc.sync.dma_start(bqk[:], bqk_d[:])
            fc1t = pw.tile([128, 2, 16], f32)
            nc.sync.dma_start(fc1t[:], fc1t_d[:].rearrange("j p m -> p j m"))
            fc1b = pw.tile([16, 1], f32)
            nc.sync.dma_start(fc1b[:], fc1b_d[:])
            fc2t = pw.tile([16, 2, 128], f32)
            nc.sync.dma_start(fc2t[:], fc2t_d[:].rearrange("m k f -> k m f"))
            fc2bn = pw.tile([128, 2, 1], f32)
            nc.sync.dma_start(fc2bn[:], fc2bn_d[:].rearrange("m p o -> p m o"))
            fuseb = pw.tile([128, 2, 1], f32)
            nc.sync.dma_start(fuseb[:], fuseb_d[:].rearrange("m p o -> p m o"))

            qk = pw.tile([128, HW], f32r)         # q rows 0-63 (cols 0-2047), k rows 64-127
            q64 = pw.tile([128, HWh], f32r)       # q copy at partitions 64-127
            klo = pw.tile([64, HW], f32r)         # k copy at partitions 0-63
            vt = pw.tile([128, JT, 256], f32r)    # vT[j, c]
            glob = [pw.tile([128, 32, 64], f32r, tag=f"glob{t}", name=f"glob{t}") for t in range(CT)]
            yse = [pw.tile([128, 1], f32, tag=f"yse{t}", name=f"yse{t}") for t in range(CT)]

            with tc.tile_pool(name="px", bufs=1) as px, \
                 tc.tile_pool(name="psqk", bufs=2, space="PSUM") as psqk, \
                 tc.tile_pool(name="psvt", bufs=2, space="PSUM") as psvt, \
                 tc.tile_pool(name="psse", bufs=1, space="PSUM") as psse:
                xp = [px.tile([128, 68, 66], f32r, tag=f"xp{j}", name=f"xp{j}") for j in range(CT)]
                for j in range(CT):
                    # two DMAs per ci-tile: segment A (rows 0-33), segment B (34-67)
                    src = xp_d[j * 128:(j + 1) * 128, :].rearrange(
                        "p (h w) -> p h w", w=66)
                    nc.sync.dma_start(xp[j][:, 0:34, :], src[:, 0:34, :])
                    nc.sync.dma_start(xp[j][:, 34:68, :], src[:, 34:68, :])

                # ---- SE channel sums (mean folded into fc1 weights host-side)
                sums = [pw.tile([128, 1], f32, tag=f"sums{j}", name=f"sums{j}") for j in range(CT)]
                sa = pw.tile([128, 1], f32)
                sb_ = pw.tile([128, 1], f32)
                for j in range(CT):
                    nc.vector.reduce_sum(sa[:], xp[j][:, 1:33, 1:65].bitcast(f32),
                                         axis=mybir.AxisListType.XY)
                    nc.vector.reduce_sum(sb_[:], xp[j][:, 35:67, 1:65].bitcast(f32),
                                         axis=mybir.AxisListType.XY)
                    nc.vector.tensor_add(sums[j][:], sa[:], sb_[:])

                # ---- SE MLP: y = sigmoid(fc2 @ relu(fc1 @ mean + b1) + b2)
                ps1 = psse.tile([16, 1], f32)
                for j in range(CT):
                    nc.tensor.matmul(ps1[:], fc1t[:, j, :], sums[j][:],
                                     start=(j == 0), stop=(j == CT - 1))
                y1 = pw.tile([16, 1], f32)
                nc.scalar.activation(y1[:], ps1[:], AF.Relu, bias=fc1b[:])
                for t in range(CT):
                    ps2 = psse.tile([128, 1], f32, tag="ps2")
                    nc.tensor.matmul(ps2[:], fc2t[:, t, :], y1[:],
                                     start=True, stop=True)
                    # sigmoid(z) = 1/(1+exp(-z)), z = ps2 + fc2b ; fc2bn = -fc2b
                    en = pw.tile([128, 1], f32, tag="en")
                    nc.scalar.activation(en[:], ps2[:], AF.Exp,
                                         bias=fc2bn[:, t, :], scale=-1.0)
                    nc.vector.tensor_scalar_add(en[:], en[:], 1.0)
                    nc.vector.reciprocal(yse[t][:], en[:])

                # ---- fused q+k conv (co=128, full rolled image)
                for c in range(8):
                    base = c * 8 if c < 4 else 34 + (c - 4) * 8
                    pqk = psqk.tile([128, 512], f32)
                    first = True
                    for j in range(CT):
                        for dy in range(3):
                            for dx in range(3):
                                t = (dy * 3 + dx) * 2 + j
                                nc.tensor.matmul(
                                    pqk[:],
                                    wqkt[:, t, :],
                                    xp[j][:, base + dy:base + dy + 8, dx:dx + 64],
                                    start=first,
                                    stop=(t == 17),
                                )
                                first = False
                    nc.scalar.activation(
                        qk[:, c * 512:(c + 1) * 512].rearrange(
                            "p (h w) -> p h w", w=64),
                        pqk[:].rearrange("p (h w) -> p h w", w=64),
                        AF.Identity, bias=bqk[:])

                # ---- v conv (normal layout), then PE-transpose into vt[j, c]
                ident = pw.tile([128, 128], f32)
                make_identity(nc, ident)
                v_sb = [px.tile([128, HW], f32, tag=f"vsb{t}", name=f"vsb{t}")
                        for t in range(CT)]
                for ct in range(CT):
                    for c in range(8):
                        base = c * 8 if c < 4 else 34 + (c - 4) * 8
                        pv = psvt.tile([128, 512], f32, tag="pv", name="pv")
                        first = True
                        for j in range(CT):
                            for dy in range(3):
                                for dx in range(3):
                                    t = (dy * 3 + dx) * 2 + j
                                    nc.tensor.matmul(
                                        pv[:],
                                        wvt[:, t, ct * 128:(ct + 1) * 128],
                                        xp[j][:, base + dy:base + dy + 8, dx:dx + 64],
                                        start=first,
                                        stop=(t == 17),
                                    )
                                    first = False
                        nc.vector.tensor_copy(v_sb[ct][:, c * 512:(c + 1) * 512],
                                              pv[:])
                for ct in range(CT):
                    for jt in range(JT):
                        ptr = psvt.tile([128, 128], f32, tag="ptr", name="ptr")
                        nc.tensor.transpose(ptr[:], v_sb[ct][:, jt * 128:(jt + 1) * 128],
                                            ident[:])
                        nc.vector.tensor_copy(vt[:, jt, ct * 128:(ct + 1) * 128],
                                              ptr[:])

                # ---- glob = x_half * yse  (before xp pool closes)
                for t in range(CT):
                    nc.vector.tensor_scalar_mul(glob[t][:],
                                                xp[t][:, 1:33, 1:65].bitcast(f32),
                                                yse[t][:, 0:1])

                # q/k copies for row-tiled score matmuls
                nc.sync.dma_start(q64[64:128, :], qk[0:64, 0:HWh])
                nc.sync.dma_start(klo[:], qk[64:128, :])

            # ---- attention ----
            with tc.tile_pool(name="pa", bufs=1) as pa, \
                 tc.tile_pool(name="pet", bufs=3) as pet, \
                 tc.tile_pool(name="po", bufs=3) as po, \
                 tc.tile_pool(name="psT", bufs=2, space="PSUM") as psT, \
                 tc.tile_pool(name="psU", bufs=1, space="PSUM") as psU, \
                 tc.tile_pool(name="psF", bufs=2, space="PSUM") as psF:
                dacc = pa.tile([128, HWh], f32r)
                usb = [pa.tile([128, HWh], f32, tag=f"usb{t}", name=f"usb{t}") for t in range(CT)]
                loc = [pa.tile([128, HWh], f32r, tag=f"loc{t}", name=f"loc{t}") for t in range(CT)]
                recb = pa.tile([128, HWh], f32)

                for ih in range(IH):
                    i0 = ih * 1024
                    pu = [psU.tile([128, 512], f32, tag=f"pu{t}{icq}", name=f"pu{t}{icq}")
                          for t in range(CT) for icq in range(ICH)]
                    for jt in range(JT):
                        et = pet.tile([128, 1024], f32r, tag="et")
                        for icq in range(ICH):
                            pT = psT.tile([128, 512], f32, tag="pT")
                            isl = slice(i0 + icq * 512, i0 + (icq + 1) * 512)
                            if jt % 2 == 0:
                                nc.tensor.matmul(pT[:],
                                                 klo[:, jt * 128:(jt + 1) * 128],
                                                 qk[0:64, isl],
                                                 start=True, stop=True)
                            else:
                                nc.tensor.matmul(pT[:],
                                                 qk[64:128, jt * 128:(jt + 1) * 128],
                                                 q64[64:128, isl],
                                                 start=True, stop=True)
                            nc.scalar.activation(
                                et[:, icq * 512:(icq + 1) * 512], pT[:], AF.Exp)
                        for t in range(CT):
                            for icq in range(ICH):
                                nc.tensor.matmul(
                                    pu[t * ICH + icq][:],
                                    vt[:, jt, t * 128:(t + 1) * 128],
                                    et[:, icq * 512:(icq + 1) * 512],
                                    start=(jt == 0), stop=(jt == JT - 1),
                                )
                        if jt == 0:
                            nc.vector.tensor_copy(dacc[:, i0:i0 + 1024], et[:])
                        else:
                            nc.vector.tensor_add(dacc[:, i0:i0 + 1024],
                                                 dacc[:, i0:i0 + 1024],
                                                 et[:])
                    for t in range(CT):
                        for icq in range(ICH):
                            isl = slice(i0 + icq * 512, i0 + (icq + 1) * 512)
                            nc.vector.tensor_copy(usb[t][:, isl],
                                                  pu[t * ICH + icq][:])

                # ---- denominators: ones-matmul partition reduction + reciprocal
                onecf = pa.tile([128, 1], f32)
                nc.vector.memset(onecf[:], 1.0)
                onec = pa.tile([128, 1], f32r)
                nc.vector.tensor_copy(onec[:], onecf[:])
                dr = pa.tile([1, HWh], f32r)
                for icq in range(4):
                    pD = psT.tile([128, 512], f32, tag="pT", name="pD")
                    nc.tensor.matmul(pD[0:1, :], onec[:],
                                     dacc[:, icq * 512:(icq + 1) * 512],
                                     start=True, stop=True)
                    nc.vector.reciprocal(dr[:, icq * 512:(icq + 1) * 512],
                                         pD[0:1, :])
                onerf = pa.tile([1, 128], f32)
                nc.vector.memset(onerf[:], 1.0)
                oner = pa.tile([1, 128], f32r)
                nc.vector.tensor_copy(oner[:], onerf[:])
                for icq in range(4):
                    pB = psT.tile([128, 512], f32, tag="pT")
                    nc.tensor.matmul(pB[:], oner[:],
                                     dr[:, icq * 512:(icq + 1) * 512],
                                     start=True, stop=True)
                    nc.vector.tensor_copy(recb[:, icq * 512:(icq + 1) * 512],
                                          pB[:])

                if debug:
                    nc.sync.dma_start(qk_dbg[:], qk[:].bitcast(f32))
                    nc.sync.dma_start(vt_dbg[:],
                                      vt[:].rearrange("p a b -> p (a b)").bitcast(f32))
                    nc.sync.dma_start(dacc_dbg[:], dacc[:].bitcast(f32))
                    nc.sync.dma_start(recb_dbg[:], recb[:])
                    for t in range(CT):
                        nc.sync.dma_start(usb_dbg[t], usb[t][:])
                        nc.sync.dma_start(
                            glob_dbg[t],
                            glob[t][:].rearrange("p a b -> p (a b)").bitcast(f32))

                # ---- local = U / denom
                for t in range(CT):
                    nc.vector.tensor_mul(loc[t][:], usb[t][:], recb[:])

                # ---- fuse 1x1 conv + bias (bv and fuse_b folded host-side)
                for mt in range(CT):
                    for icq in range(4):
                        isl = slice(icq * 512, (icq + 1) * 512)
                        pf = psF.tile([128, 512], f32, tag="pf")
                        for kt in range(4):
                            rhs = (loc[kt][:, isl] if kt < 2 else
                                   glob[kt - 2][:, icq * 8:(icq + 1) * 8, :])
                            nc.tensor.matmul(pf[:], fuset[:, kt * 2 + mt, :], rhs,
                                             start=(kt == 0), stop=(kt == 3))
                        ob = po.tile([128, 512], f32, tag="ob")
                        nc.scalar.activation(ob[:], pf[:], AF.Identity,
                                             bias=fuseb[:, mt, :])
                        nc.sync.dma_start(out_d[mt, :, isl], ob[:])

    nc.compile()
    return nc


def _prep_core_inputs(inputs):
    x = np.ascontiguousarray(inputs["x"], np.float32)
    wq = np.asarray(inputs["wq"], np.float32)
    bq = np.asarray(inputs["bq"], np.float32)
    wk = np.asarray(inputs["wk"], np.float32)
    bk = np.asarray(inputs["bk"], np.float32)
    wv = np.asarray(inputs["wv"], np.float32)
    bv = np.asarray(inputs["bv"], np.float32)
    fc1_w = np.asarray(inputs["fc1_w"], np.float32)
    fc1_b = np.asarray(inputs["fc1_b"], np.float32)
    fc2_w = np.asarray(inputs["fc2_w"], np.float32)
    fc2_b = np.asarray(inputs["fc2_b"], np.float32)
    fuse_w = np.asarray(inputs["fuse_w"], np.float32)[:, :, 0, 0]
    fuse_b = np.asarray(inputs["fuse_b"], np.float32)

    scale = np.float32(Cr ** -0.5)
    wqk = np.concatenate([wq * scale, wk], axis=0)          # [128, 256, 3, 3]
    bqk = np.concatenate([bq * scale, bk])[:, None].astype(np.float32)

    wqkt = np.empty((18, 128, 128), np.float32)
    wvt = np.empty((18, 128, 256), np.float32)
    for dy in range(3):
        for dx in range(3):
            for j in range(CT):
                t = (dy * 3 + dx) * 2 + j
                wqkt[t] = wqk[:, j * 128:(j + 1) * 128, dy, dx].T
                wvt[t] = wv[:, j * 128:(j + 1) * 128, dy, dx].T

    fc1t = np.stack([(fc1_w / HW)[:, j * 128:(j + 1) * 128].T for j in range(CT)])
    fc1b = fc1_b[:, None].astype(np.float32)
    fc2t = np.stack([fc2_w[t * 128:(t + 1) * 128, :].T for t in range(CT)])
    fc2bn = np.stack([-fc2_b[t * 128:(t + 1) * 128, None] for t in range(CT)])

    fuse_b_eff = fuse_b + fuse_w[:, :C] @ bv
    fuset = np.empty((4, 2, 128, 128), np.float32)
    for kt in range(4):
        for mt in range(CT):
            fuset[kt, mt] = fuse_w[mt * 128:(mt + 1) * 128,
                                   kt * 128:(kt + 1) * 128].T
    fuseb = np.stack([fuse_b_eff[t * 128:(t + 1) * 128, None] for t in range(CT)])

    shared = dict(
        wqkt=wqkt, wvt=wvt, bqk=bqk,
        fc1t=np.ascontiguousarray(fc1t), fc1b=fc1b,
        fc2t=np.ascontiguousarray(fc2t), fc2bn=np.ascontiguousarray(fc2bn),
        fuset=fuset, fuseb=np.ascontiguousarray(fuseb),
    )

    in_maps = []
    for core in range(N_CORES):
        s, p = divmod(core, 2)
        s0 = p * 32
        t0 = (s0 + 32) % 64
        P = np.zeros((C, 66, 66), np.float32)
        P[:, 1:65, 1:65] = x[s]
        xp = np.concatenate([P[:, s0:s0 + 34], P[:, t0:t0 + 34]], axis=1)
        m = dict(shared)
        m["xp"] = np.ascontiguousarray(xp.reshape(C, 68 * 66))
        in_maps.append(m)
    return in_maps


def kernel(**inputs):
    global _compiled
    if _compiled is None:
        _compiled = _build()
    nc = _compiled
    in_maps = _prep_core_inputs(inputs)
    res = run_bass_kernel_spmd(nc, in_maps, list(range(N_CORES)))
    out = np.empty((4, C, H, W), np.float32)
    for core in range(N_CORES):
        s, p = divmod(core, 2)
        o = res.results[core]["out"]          # [2, 128, 2048]
        out[s, :, p * 32:(p + 1) * 32, :] = o.reshape(C, 32, 64)
    return out
